# revision 9
# baseline (speedup 1.0000x reference)
"""Trainium2 Bass kernel for nn_Attention_90228672954441.

Spatial-reduction attention (PVT-style), computed twice (x0 with ln0, x1 with
ln1). Reference math per input x (B=2, N=4096, C=256):
  q = x @ Wq.T                                   -> (B, N, C), heads h=8, d=32
  xs = conv2x2_s2(x as NCHW 64x64, Wsr) + bsr    -> (B, M=1024, C)
  xs = layernorm(xs, ln_w, ln_b)
  k, v = split(xs @ Wkv.T)                       -> (B, h, M, d)
  attn = softmax(q k^T / sqrt(d)); out = attn @ v
  y = out @ Wproj.T + bproj

Sharding (8 cores, no collectives): core = (input i, batch b, query-half).
Each core computes y.T for its 2048 query rows completely. The conv/LN/kv
stage (tiny) is duplicated between the 2 cores of an (i, b) pair.

On-device layout: everything flows channel-major ("transposed") so all
matmuls contract over partitions: x.T (C,N) -> q.T, k.T ((h d), M),
v (M, (h d)), S.T (m-part, n-free) per head. Softmax over keys m (the
partition dim of S.T) uses unnormalized exp on ACT (logits are ~N(0, 0.01),
so no max subtraction is needed for fp32 safety), the denominator Z comes
from an all-ones matmul on the TensorE (replicated across 32 partitions so it
aligns with O'.T = V.T P.T), and normalization happens on the small O' (256 x
2048) instead of the big P (8 x 2048 x 1024).
"""

import numpy as np

B, N, C, HEADS, SR = 2, 4096, 256, 8, 2
HW = 64
D = C // HEADS           # 32
M = (HW // SR) ** 2      # 1024
NCORES = 8
NHALF = N // 2           # 2048 query rows per core
P = 128
KO = C // P              # 2 contraction subtiles over channels
NCH = NHALF // 512       # 4 n-chunks of 512
MT = M // P              # 8 m-tiles
SCALE = float(D) ** -0.5


def build_nc():
    import concourse.bacc as bacc
    import concourse.bass as bass
    import concourse.mybir as mybir
    import concourse.tile as tile
    from concourse.masks import make_identity

    fp32 = mybir.dt.float32
    AF = mybir.ActivationFunctionType

    nc = bacc.Bacc(None, target_bir_lowering=False)

    # xt is host-side im2col'd: xt[ci, khw*M + m] = x[n(m, khw), ci]
    xt_d = nc.dram_tensor("xt", [C, SR * SR * M], fp32, kind="ExternalInput")
    xq_d = nc.dram_tensor("xq", [C, NHALF], fp32, kind="ExternalInput")
    wq_d = nc.dram_tensor("wq_t", [C, C], fp32, kind="ExternalInput")
    wk_d = nc.dram_tensor("wk_t", [C, C], fp32, kind="ExternalInput")
    wv_d = nc.dram_tensor("wv_t", [C, C], fp32, kind="ExternalInput")
    wp_d = nc.dram_tensor("wp_t", [C, C], fp32, kind="ExternalInput")
    wsr_d = nc.dram_tensor("wsr_t", [C, SR * SR * C], fp32, kind="ExternalInput")
    bsr_d = nc.dram_tensor("bsr", [C], fp32, kind="ExternalInput")
    bp_d = nc.dram_tensor("bproj", [C], fp32, kind="ExternalInput")
    lnw_d = nc.dram_tensor("ln_w", [C], fp32, kind="ExternalInput")
    lnb_d = nc.dram_tensor("ln_b", [C], fp32, kind="ExternalInput")
    yt_d = nc.dram_tensor("yt", [C, NHALF], fp32, kind="ExternalOutput")

    xt_r = xt_d.rearrange("(ko p) (k m) -> p ko k m", p=P, m=M)
    xq_r = xq_d.rearrange("(ko p) n -> p ko n", p=P)

    with tile.TileContext(nc) as tc:
        with (
            tc.tile_pool(name="consts", bufs=1) as consts,
            tc.tile_pool(name="persist", bufs=1) as persist,
            tc.tile_pool(name="stream", bufs=3) as stream,
            tc.tile_pool(name="pt", bufs=6) as ptpool,
            tc.tile_pool(name="small", bufs=3) as small,
            tc.tile_pool(name="stps", bufs=3, space="PSUM") as stps,
            tc.tile_pool(name="accps", bufs=1, space="PSUM") as accps,
        ):
            # ---- constants / weights in SBUF ----
            wq_sb = consts.tile([P, KO, C], fp32, tag="wq")
            nc.sync.dma_start(wq_sb[:], wq_d.rearrange("(ko p) o -> p ko o", p=P))
            wk_sb = consts.tile([P, KO, C], fp32, tag="wk")
            nc.sync.dma_start(wk_sb[:], wk_d.rearrange("(ko p) o -> p ko o", p=P))
            wv_sb = consts.tile([P, KO, C], fp32, tag="wv")
            nc.sync.dma_start(wv_sb[:], wv_d.rearrange("(ko p) o -> p ko o", p=P))
            wp_sb = consts.tile([P, KO, C], fp32, tag="wp")
            nc.sync.dma_start(wp_sb[:], wp_d.rearrange("(ko p) o -> p ko o", p=P))
            wsr_sb = consts.tile([P, KO, SR * SR, C], fp32, tag="wsr")
            wsr_r = wsr_d.rearrange("(ko p) (k o) -> p ko k o", p=P, o=C)
            for ko in range(KO):
                nc.sync.dma_start(wsr_sb[:, ko], wsr_r[:, ko])

            def bcast_load(dram_h, tag):
                t = consts.tile([P, C], fp32, tag=tag)
                src = dram_h[:]
                bc = bass.AP(tensor=src.tensor, offset=src.offset,
                             ap=[[0, P]] + list(src.ap))
                nc.gpsimd.dma_start(out=t[:], in_=bc)
                return t

            bsr_sb = bcast_load(bsr_d, "bsr")     # [128, 256] replicated rows
            lnw_sb = bcast_load(lnw_d, "lnw")
            lnb_sb = bcast_load(lnb_d, "lnb")
            bp_sb = consts.tile([P, KO], fp32, tag="bp")  # per-partition bias
            nc.sync.dma_start(bp_sb[:], bp_d.rearrange("(ko p) -> p ko", p=P))

            eps_sb = consts.tile([P, 1], fp32, tag="eps")
            nc.vector.memset(eps_sb[:], 1e-5)
            ones_sb = consts.tile([P, D], fp32, tag="ones")
            nc.vector.memset(ones_sb[:], 1.0)
            ident = consts.tile([P, P], fp32, tag="ident")
            make_identity(nc, ident[:])

            # ---- persistent activations ----
            qt_sb = persist.tile([P, KO, NHALF], fp32, tag="qt")   # q.T
            xs_sb = persist.tile([P, MT, C], fp32, tag="xs")       # xs (m, c)
            xst_sb = persist.tile([P, KO, M], fp32, tag="xst")     # xs.T
            kt_sb = persist.tile([P, KO, M], fp32, tag="kt")       # k.T
            v_sb = persist.tile([P, MT, C], fp32, tag="v")         # v (m, hd)
            ot_sb = persist.tile([P, KO, NHALF], fp32, tag="ot")   # O.T

            # ---- Phase B: q.T = Wq @ x.T (this core's query half) ----
            for nch in range(NCH):
                xqt = stream.tile([P, KO, 512], fp32, tag="xq")
                nc.sync.dma_start(xqt[:], xq_r[:, :, nch * 512:(nch + 1) * 512])
                for ot in range(KO):
                    ps = stps.tile([P, KO, 512], fp32, tag="st")
                    for ko in range(KO):
                        nc.tensor.matmul(
                            ps[:, 0, :],
                            wq_sb[:, ko, ot * P:(ot + 1) * P],
                            xqt[:, ko, :],
                            start=(ko == 0), stop=(ko == KO - 1),
                        )
                    nc.vector.tensor_copy(
                        qt_sb[:, ot, nch * 512:(nch + 1) * 512], ps[:, 0, :]
                    )

            # ---- Phase C: conv (2x2 stride 2) + bias + layernorm ----
            for mt in range(MT):
                xtile = stream.tile([P, KO, SR * SR, P], fp32, tag="xc")
                for ko in range(KO):
                    nc.sync.dma_start(xtile[:, ko],
                                      xt_r[:, ko, :, mt * P:(mt + 1) * P])
                ps = stps.tile([P, KO, 512], fp32, tag="st")
                first = True
                for ko in range(KO):
                    for k in range(SR * SR):
                        nc.tensor.matmul(
                            ps[:, 0, :C],
                            xtile[:, ko, k, :],
                            wsr_sb[:, ko, k, :],
                            start=first,
                            stop=(ko == KO - 1 and k == SR * SR - 1),
                        )
                        first = False
                nc.vector.tensor_add(xs_sb[:, mt, :], ps[:, 0, :C], bsr_sb[:])
                # layernorm over free dim (C)
                stats = small.tile([P, 6], fp32, tag="stats")
                nc.vector.bn_stats(out=stats[:], in_=xs_sb[:, mt, :])
                mv = small.tile([P, 2], fp32, tag="mv")
                nc.vector.bn_aggr(out=mv[:], in_=stats[:])
                rstd = small.tile([P, 1], fp32, tag="rstd")
                nc.scalar.activation(rstd[:], mv[:, 1:2], AF.Sqrt, bias=eps_sb[:])
                nc.vector.reciprocal(rstd[:], rstd[:])
                nc.vector.tensor_scalar(
                    xs_sb[:, mt, :], xs_sb[:, mt, :],
                    scalar1=mv[:, 0:1], scalar2=rstd[:],
                    op0=mybir.AluOpType.subtract, op1=mybir.AluOpType.mult,
                )
                nc.vector.tensor_mul(xs_sb[:, mt, :], xs_sb[:, mt, :], lnw_sb[:])
                nc.vector.tensor_add(xs_sb[:, mt, :], xs_sb[:, mt, :], lnb_sb[:])

            # ---- Phase D: xs.T via PE transpose ----
            for mt in range(MT):
                for ct in range(KO):
                    tp = stps.tile([P, KO, 512], fp32, tag="st")
                    nc.tensor.transpose(
                        tp[:, 0, :P], xs_sb[:, mt, ct * P:(ct + 1) * P], ident[:]
                    )
                    nc.vector.tensor_copy(
                        xst_sb[:, ct, mt * P:(mt + 1) * P], tp[:, 0, :P]
                    )

            # ---- Phase E: k.T = Wk @ xs.T ----
            for hdt in range(KO):
                for mch in range(M // 512):
                    ps = stps.tile([P, KO, 512], fp32, tag="st")
                    for ko in range(KO):
                        nc.tensor.matmul(
                            ps[:, 0, :],
                            wk_sb[:, ko, hdt * P:(hdt + 1) * P],
                            xst_sb[:, ko, mch * 512:(mch + 1) * 512],
                            start=(ko == 0), stop=(ko == KO - 1),
                        )
                    nc.vector.tensor_copy(
                        kt_sb[:, hdt, mch * 512:(mch + 1) * 512], ps[:, 0, :]
                    )

            # ---- Phase F: v = xs @ Wv.T (natural (m, hd) layout) ----
            for mt in range(MT):
                ps = stps.tile([P, KO, 512], fp32, tag="st")
                for ko in range(KO):
                    nc.tensor.matmul(
                        ps[:, 0, :C],
                        xst_sb[:, ko, mt * P:(mt + 1) * P],
                        wv_sb[:, ko, :],
                        start=(ko == 0), stop=(ko == KO - 1),
                    )
                nc.vector.tensor_copy(v_sb[:, mt, :], ps[:, 0, :C])

            # ---- Phase G: attention ----
            # Head pairs hp: heads (2hp, 2hp+1); hdt = hp//2; row pos j = h%4.
            for nch in range(NCH):
                nsl = slice(nch * 512, (nch + 1) * 512)
                for hp in range(HEADS // 2):
                    hdt = hp // 2
                    o_ps = accps.tile([2 * D, 512], fp32, tag="ops")
                    z_ps = accps.tile([2 * D, 512], fp32, tag="zps")
                    pts = [None] * MT
                    for mt in range(MT):
                        st = stps.tile([P, KO, 512], fp32, tag="st")
                        for jj in range(2):
                            j = (hp % 2) * 2 + jj
                            nc.tensor.matmul(
                                st[:, jj, :],
                                kt_sb[32 * j:32 * (j + 1), hdt,
                                      mt * P:(mt + 1) * P],
                                qt_sb[32 * j:32 * (j + 1), hdt, nsl],
                                start=True, stop=True,
                                tile_position=(32 * j, 0),
                            )
                        pt = ptpool.tile([P, KO, 512], fp32, tag="pt")
                        nc.scalar.activation(pt[:], st[:], AF.Exp, scale=SCALE)
                        pts[mt] = pt
                        # PV + Z for the previous m-tile keeps PE behind ACT
                        for pv_mt in ([mt - 1] if mt > 0 else []) + (
                            [mt] if mt == MT - 1 else []
                        ):
                            pvt = pts[pv_mt]
                            for jj in range(2):
                                h = 2 * hp + jj
                                # skip_group_check: the two col-groups share a
                                # PSUM bank but write disjoint partition rows;
                                # has_written is per-element on HW.
                                nc.tensor.matmul(
                                    o_ps[32 * jj:32 * (jj + 1), :],
                                    v_sb[:, pv_mt, h * D:(h + 1) * D],
                                    pvt[:, jj, :],
                                    start=(pv_mt == 0), stop=(pv_mt == MT - 1),
                                    tile_position=(0, 32 * jj),
                                    skip_group_check=True,
                                )
                                nc.tensor.matmul(
                                    z_ps[32 * jj:32 * (jj + 1), :],
                                    ones_sb[:],
                                    pvt[:, jj, :],
                                    start=(pv_mt == 0), stop=(pv_mt == MT - 1),
                                    tile_position=(0, 32 * jj),
                                    skip_group_check=True,
                                )
                    # normalize O' by Z and store to O.T
                    zr = small.tile([2 * D, 512], fp32, tag="zr")
                    nc.vector.reciprocal(zr[:], z_ps[:])
                    pbase = 64 * (hp % 2)
                    nc.vector.tensor_mul(
                        ot_sb[pbase:pbase + 2 * D, hdt, nsl], o_ps[:], zr[:]
                    )

            # ---- Phase H: y.T = Wproj @ O.T + bproj ----
            for nch in range(NCH):
                nsl = slice(nch * 512, (nch + 1) * 512)
                for ot in range(KO):
                    ps = stps.tile([P, KO, 512], fp32, tag="st")
                    for ct in range(KO):
                        nc.tensor.matmul(
                            ps[:, 0, :],
                            wp_sb[:, ct, ot * P:(ot + 1) * P],
                            ot_sb[:, ct, nsl],
                            start=(ct == 0), stop=(ct == KO - 1),
                        )
                    yt_t = stream.tile([P, 512], fp32, tag="yt")
                    nc.vector.tensor_scalar_add(yt_t[:], ps[:, 0, :],
                                                bp_sb[:, ot:ot + 1])
                    nc.sync.dma_start(yt_d[ot * P:(ot + 1) * P, nsl], yt_t[:])

    return nc


def _prep_core_inputs(x_np, Wq, Wkv, Wproj, bproj, Wsr, bsr, ln_w, ln_b):
    """Host-side shard prep shared by all cores of one (input, batch) pair."""
    f = np.float32
    wq_t = np.ascontiguousarray(Wq.T, dtype=f)
    wk_t = np.ascontiguousarray(Wkv[:C].T, dtype=f)
    wv_t = np.ascontiguousarray(Wkv[C:].T, dtype=f)
    wp_t = np.ascontiguousarray(Wproj.T, dtype=f)
    # (ci, kh, kw, o) flattened to (ci, kh*kw*o): per-ci row is contiguous
    wsr_t = np.ascontiguousarray(
        Wsr.transpose(1, 2, 3, 0).reshape(C, SR * SR * C), dtype=f
    )
    return {
        "wq_t": wq_t, "wk_t": wk_t, "wv_t": wv_t, "wp_t": wp_t,
        "wsr_t": wsr_t,
        "bsr": np.ascontiguousarray(bsr, dtype=f),
        "bproj": np.ascontiguousarray(bproj, dtype=f),
        "ln_w": np.ascontiguousarray(ln_w, dtype=f),
        "ln_b": np.ascontiguousarray(ln_b, dtype=f),
    }


def kernel(x0, x1, Wq, Wkv, Wproj, bproj, Wsr, bsr, ln_w0, ln_b0,
           ln_w1, ln_b1, H, W):
    from concourse.bass_utils import run_bass_kernel_spmd

    assert int(H) == HW and int(W) == HW
    x0 = np.asarray(x0, dtype=np.float32)
    x1 = np.asarray(x1, dtype=np.float32)

    common = [
        _prep_core_inputs(None, np.asarray(Wq), np.asarray(Wkv),
                          np.asarray(Wproj), np.asarray(bproj),
                          np.asarray(Wsr), np.asarray(bsr),
                          np.asarray(lw), np.asarray(lb))
        for (lw, lb) in ((ln_w0, ln_b0), (ln_w1, ln_b1))
    ]

    in_maps = []
    for c in range(NCORES):
        i, b, half = c // 4, (c // 2) % 2, c % 2
        x = x0 if i == 0 else x1
        xt = np.ascontiguousarray(x[b].T, dtype=np.float32)       # (C, N)
        xq = np.ascontiguousarray(xt[:, half * NHALF:(half + 1) * NHALF])
        # im2col for the 2x2/s2 conv: (C, (kh, kw), (ih, iw))
        xg = np.ascontiguousarray(
            xt.reshape(C, 32, 2, 32, 2).transpose(0, 2, 4, 1, 3)
              .reshape(C, SR * SR * M))
        m = dict(common[i])
        m["xt"] = xg
        m["xq"] = xq
        in_maps.append(m)

    nc = build_nc()
    nc.finalize()
    res = run_bass_kernel_spmd(nc, in_maps, core_ids=list(range(NCORES)))

    y = np.zeros((2, B, N, C), dtype=np.float32)
    for c in range(NCORES):
        i, b, half = c // 4, (c // 2) % 2, c % 2
        y[i, b, half * NHALF:(half + 1) * NHALF, :] = res.results[c]["yt"].T
    return y


if __name__ == "__main__":
    pass
